# revision 11
# baseline (speedup 1.0000x reference)
"""Multi-head self-attention (B=4, N=2048, C=1024, H=16) on 4 Trainium2 cores.

v2 design, driven by measurement: per-execution cost on this axon-tunneled
setup is dominated by STAGING of declared input params + outputs (~6-12 GB/s
aggregate), not by compute. So:
  - 4 cores, one batch each (no input duplication across cores).
  - Weights ship as inline NEFF constants (staged once at model load, not
    per execution). Only x streams per execution (bf16 [1024, 2048] per core)
    and the final output returns as f16 (device adds b_proj; no host combine).
  - Per core: two sequential head-group passes (8 heads each) reusing SBUF
    buffers; device-side layout identical to the proven v1 kernel
    (S^T-layout softmax, ones-column row sums, exp on ACT, O^T proj).
"""

import os
import sys

if "/opt/trn_rl_repo" not in sys.path:
    sys.path.insert(0, "/opt/trn_rl_repo")

if "axon" not in os.environ.get("JAX_PLATFORMS", "axon"):
    os.environ["JAX_PLATFORMS"] = "axon"

from contextlib import ExitStack

import ml_dtypes
import numpy as np

import concourse.bass as bass
import concourse.tile as tile
from concourse import mybir

B, N, C = 4, 2048, 1024
H, DH = 16, 64
G = 2                 # head-group passes per core
HG = 8                # heads per group
HD = HG * DH          # 512 head-dims per group
SCALE = DH ** -0.5
KT = 9                # contraction k-tiles for V matmul (8 x + 1 bias/ones)
NCORES = 4

F32 = mybir.dt.float32
F16 = mybir.dt.float16
DT = mybir.dt.bfloat16
NPDT = ml_dtypes.bfloat16


def _replace_sem_range_clear(nc):
    """Replace the EVENT_SEMAPHORE_RANGE_CLEAR that TileContext emits (and
    this walrus build rejects) with per-semaphore sem-wr-imm zero writes."""
    f = nc.m.functions[0]
    blocks = list(f.blocks)
    snaps = [list(b.instructions) for b in blocks]
    totals = {}
    for insts in snaps:
        for i in insts:
            si = i.sync_info
            if si:
                for u in si.on_update:
                    if u.sync_type == "semaphore":
                        totals[u.id] = totals.get(u.id, 0) + u.update_value
    newlists = []
    for insts in snaps:
        newlist = []
        for i in insts:
            if type(i).__name__ == "InstISA" and "RANGE_CLEAR" in (i.op_name or ""):
                d = i.ant_dict
                for sem in range(d["range_first"], d["range_last"] + 1):
                    v = totals.get(sem, 0)
                    if v == 0:
                        continue
                    car = mybir.InstEventSemaphore(
                        name=nc.get_next_instruction_name()
                    )
                    car.engine = i.engine
                    car.sync_info = mybir.SyncInfo(
                        on_wait=[],
                        on_update=[
                            mybir.SyncUpdate(
                                sync_type="semaphore",
                                id=sem,
                                update_mode="sem-wr-imm",
                                update_value=0,
                                update_reg=None,
                            )
                        ],
                    )
                    newlist.append(car)
                continue
            newlist.append(i)
        newlists.append(newlist)
    for b, nl in zip(blocks, newlists):
        b.instructions = nl


def _split_multi_waits(nc):
    """Walrus allows one sync wait per instruction; hoist extras onto cheap
    same-engine carrier instructions placed immediately before. Matmul syncs
    ride the paired LDWEIGHTS' single slot, so matmuls keep zero waits."""
    def make_carrier(engine):
        car = mybir.InstEventSemaphore(name=nc.get_next_instruction_name())
        car.engine = engine
        return car

    f = nc.m.functions[0]
    blocks = list(f.blocks)
    snapshots = [list(b.instructions) for b in blocks]
    newlists = []
    for insts in snapshots:
        newlist = []
        for i in insts:
            si = i.sync_info
            ty = type(i).__name__
            if si is not None and len(si.on_wait) > 1:
                waits = list(si.on_wait)
                is_mm = ty == "InstMatmult"
                keep = 0 if is_mm else 1
                extras = waits[: len(waits) - keep]
                kept = waits[len(waits) - keep:]
                pos = len(newlist)
                if is_mm and pos > 0 and type(newlist[-1]).__name__ == "InstLdweights":
                    pos -= 1
                carriers = []
                for w in extras:
                    car = make_carrier(i.engine)
                    if car is None:
                        kept = waits
                        carriers = []
                        break
                    car.sync_info = mybir.SyncInfo(on_wait=[w], on_update=[])
                    carriers.append(car)
                if carriers or len(kept) < len(waits):
                    newlist[pos:pos] = carriers
                    i.sync_info = mybir.SyncInfo(
                        on_wait=kept, on_update=list(si.on_update)
                    )
            newlist.append(i)
        newlists.append(newlist)
    for b, nl in zip(blocks, newlists):
        b.instructions = nl


def _make_consts(w_qkv, b_qkv, w_proj, b_proj):
    """Host-side packing of the inline-const weight tensors (bf16)."""
    w_qkv = np.asarray(w_qkv, np.float32)
    b_qkv = np.asarray(b_qkv, np.float32)
    w_proj = np.asarray(w_proj, np.float32)
    b_proj = np.asarray(b_proj, np.float32)

    cqk = np.zeros((C, 2048), np.float32)
    cbqk = np.zeros((128, 16), np.float32)
    for g in range(G):
        cs = slice(512 * g, 512 * g + 512)
        wq = w_qkv[:, 0:1024][:, cs] * SCALE
        wk = w_qkv[:, 1024:2048][:, cs]
        cqk[:, g * 1024:g * 1024 + 512] = wq
        cqk[:, g * 1024 + 512:(g + 1) * 1024] = wk
        bq = b_qkv[0:1024][cs] * SCALE
        bk = b_qkv[1024:2048][cs]
        cbqk[:, g * 8:(g + 1) * 8] = (
            np.concatenate([bq, bk]).reshape(8, 128).T
        )

    cv = np.zeros((KT * 128, 1024), np.float32)
    cv[:C] = w_qkv[:, 2048:3072]
    cv[C] = b_qkv[2048:3072]

    cwp = np.zeros((KT * 128, C), np.float32)
    cwp[:C] = w_proj
    cwp[C] = b_proj

    return {
        "cqk": cqk.astype(NPDT),
        "cv": cv.astype(NPDT),
        "cwp": cwp.astype(NPDT),
        "cbqk": np.ascontiguousarray(cbqk, np.float32),
    }


def build_bass(consts, reps=1):
    """Build the kernel NEFF. With reps>1 the whole body (including the
    x DRAM->SBUF loads and the output stores) is emitted `reps` times —
    used by the harness to measure per-execution device time differentially
    (the repeated bodies run back-to-back on the device, so the fixed
    per-dispatch overhead of the tunnel is excluded)."""
    nc = bass.Bass()

    cqk = nc.inline_tensor(consts["cqk"], name="cqk")
    cv = nc.inline_tensor(consts["cv"], name="cv")
    cwp = nc.inline_tensor(consts["cwp"], name="cwp")
    cbqk = nc.inline_tensor(consts["cbqk"], name="cbqk")

    xT = nc.declare_dram_parameter("xT", [C, N], DT, isOutput=False)
    out = nc.declare_dram_parameter("out", [N, C], F16, isOutput=True)

    with tile.TileContext(nc) as tc, ExitStack() as ctx:
        res = ctx.enter_context(tc.tile_pool(name="res", bufs=1))
        ppool = ctx.enter_context(tc.tile_pool(name="ppool", bufs=5))
        spool = ctx.enter_context(tc.tile_pool(name="spool", bufs=3))
        opool = ctx.enter_context(tc.tile_pool(name="opool", bufs=2))
        ps_mm = ctx.enter_context(tc.tile_pool(name="ps_mm", bufs=2, space="PSUM"))
        ps_s = ctx.enter_context(tc.tile_pool(name="ps_s", bufs=2, space="PSUM"))
        ps_o = ctx.enter_context(tc.tile_pool(name="ps_o", bufs=2, space="PSUM"))
        dpool = ctx.enter_context(tc.tile_pool(name="dpool", bufs=4, space="DRAM"))
        wqk_pool = ctx.enter_context(tc.tile_pool(name="wqkp", bufs=1))
        wv_pool = ctx.enter_context(tc.tile_pool(name="wvp", bufs=1))
        vaug_pool = ctx.enter_context(tc.tile_pool(name="vaugp", bufs=1))
        qt_pool = ctx.enter_context(tc.tile_pool(name="qtp", bufs=1))
        kt_pool = ctx.enter_context(tc.tile_pool(name="ktp", bufs=2))

        # ---- resident SBUF tensors ----
        xT_sb = [res.tile([128, N], DT, name=f"xt{k}", tag=f"xt{k}") for k in range(8)]
        ones_sb = res.tile([128, N], DT, name="ones_sb", tag="ones_sb")
        wp_sb = [res.tile([128, C], DT, name=f"wp{t}", tag=f"wp{t}") for t in range(KT)]
        bqk_sb = res.tile([128, 16], F32, name="bqk_sb", tag="bqk_sb")
        onT_sb = [
            [res.tile([128, N], DT, name=f"ot{g}_{t}", tag=f"ot{g}_{t}") for t in range(4)]
            for g in range(G)
        ]

        def emit_g(g, rep):
            # per-group weight loads (fresh pool tiles; WAR handled by tile fw)
            wqk_sb = [
                wqk_pool.tile([128, 1024], DT, name=f"wqk{rep}_{g}_{k}", tag=f"wqk{k}")
                for k in range(8)
            ]
            wv_sb = [
                wv_pool.tile([128, HD], DT, name=f"wv{rep}_{g}_{k}", tag=f"wv{k}")
                for k in range(KT)
            ]
            vaug_sb = [
                vaug_pool.tile([128, HG, DH + 1], DT, name=f"va{rep}_{g}_{m}", tag=f"va{m}")
                for m in range(16)
            ]
            qt_sb = [
                qt_pool.tile([128, N], DT, name=f"qt{rep}_{g}_{t}", tag=f"qt{t}")
                for t in range(4)
            ]
            kt_sb = [
                kt_pool.tile([128, N], DT, name=f"kt{rep}_{g}_{t}", tag=f"kt{t}")
                for t in range(4)
            ]
            for k in range(8):
                nc.sync.dma_start(
                    out=wqk_sb[k],
                    in_=cqk[k * 128:(k + 1) * 128, g * 1024:(g + 1) * 1024],
                )
            for k in range(KT):
                nc.sync.dma_start(
                    out=wv_sb[k],
                    in_=cv[k * 128:(k + 1) * 128, g * 512:(g + 1) * 512],
                )

            def v_phase():
                for mt in range(16):
                    ps = ps_mm.tile([128, 512], F32, name=f"v_ps{rep}_{g}_{mt}", tag="mm")
                    for k in range(KT):
                        nc.tensor.matmul(
                            ps,
                            lhsT=(xT_sb[k] if k < 8 else ones_sb)[:, mt * 128:(mt + 1) * 128],
                            rhs=wv_sb[k],
                            start=(k == 0),
                            stop=(k == KT - 1),
                        )
                    va = vaug_sb[mt]
                    nc.vector.memset(va[:, :, DH:DH + 1], 1.0)
                    nc.vector.tensor_copy(
                        out=va[:, :, 0:DH],
                        in_=ps.rearrange("p (h d) -> p h d", h=HG),
                    )

            def qk_group(j, ct, copy_engine="act"):
                dst = qt_sb[ct] if ct < 4 else kt_sb[ct - 4]
                ps = ps_mm.tile([128, 512], F32, name=f"qkg_ps{rep}_{g}_{ct}_{j}", tag="mm")
                for k in range(8):
                    nc.tensor.matmul(
                        ps,
                        lhsT=wqk_sb[k][:, ct * 128:(ct + 1) * 128],
                        rhs=xT_sb[k][:, j * 512:(j + 1) * 512],
                        start=(k == 0),
                        stop=(k == 7),
                    )
                bias = bqk_sb[:, g * 8 + ct:g * 8 + ct + 1]
                if copy_engine == "act":
                    nc.scalar.activation(
                        out=dst[:, j * 512:(j + 1) * 512],
                        in_=ps,
                        func=mybir.ActivationFunctionType.Identity,
                        bias=bias,
                    )
                else:
                    nc.vector.tensor_scalar_add(
                        out=dst[:, j * 512:(j + 1) * 512],
                        in0=ps,
                        scalar1=bias,
                    )

            def attention(j, filler=None):
                nsl = slice(j * 512, (j + 1) * 512)
                for h in range(8):
                    t, pr = h // 2, (h % 2) * 64
                    po = ps_o.tile([DH + 1, 512], F32, name=f"po{rep}_{g}_{j}_{h}", tag="po")
                    pts = {}
                    for i in range(9):
                        if i < 8:
                            mtA, mtB = 2 * i, 2 * i + 1
                            ps = ps_s.tile(
                                [128, 1024], F32, name=f"s_ps{rep}_{g}_{j}_{h}_{i}", tag="ps"
                            )
                            nc.tensor.matmul(
                                ps[:, 0:512],
                                lhsT=kt_sb[t][pr:pr + 64, mtA * 128:(mtA + 1) * 128],
                                rhs=qt_sb[t][pr:pr + 64, nsl],
                                start=True,
                                stop=True,
                            )
                            nc.tensor.matmul(
                                ps[:, 512:1024],
                                lhsT=kt_sb[t][pr:pr + 64, mtB * 128:(mtB + 1) * 128],
                                rhs=qt_sb[t][pr:pr + 64, nsl],
                                start=True,
                                stop=True,
                            )
                            pt = ppool.tile(
                                [128, 1024], DT, name=f"pt{rep}_{g}_{j}_{h}_{i}", tag="pt"
                            )
                            nc.scalar.activation(
                                out=pt, in_=ps, func=mybir.ActivationFunctionType.Exp
                            )
                            pts[i] = pt
                        if i >= 1:
                            mp = i - 1
                            pt = pts.pop(mp)
                            nc.tensor.matmul(
                                po,
                                lhsT=vaug_sb[2 * mp][:, h, :],
                                rhs=pt[:, 0:512],
                                start=(mp == 0),
                                stop=False,
                            )
                            nc.tensor.matmul(
                                po,
                                lhsT=vaug_sb[2 * mp + 1][:, h, :],
                                rhs=pt[:, 512:1024],
                                start=False,
                                stop=(mp == 7),
                            )
                    o_un = spool.tile([DH + 1, 512], F32, name=f"ou{rep}_{g}_{j}_{h}", tag="oun")
                    nc.vector.tensor_copy(out=o_un, in_=po)
                    rrow = spool.tile([1, 512], F32, name=f"rr{rep}_{g}_{j}_{h}", tag="rrow")
                    nc.vector.reciprocal(out=rrow, in_=o_un[DH:DH + 1, :])
                    rdram = dpool.tile([1, 512], F32, name=f"rd{rep}_{g}_{j}_{h}", tag="rd")
                    nc.sync.dma_start(out=rdram, in_=rrow)
                    rbc = spool.tile([64, 512], F32, name=f"rb{rep}_{g}_{j}_{h}", tag="rbc")
                    bc_ap = bass.AP(
                        tensor=rdram.tensor,
                        offset=rdram.offset,
                        ap=[[0, 64]] + [list(d) for d in rdram.ap[1:]],
                    )
                    nc.sync.dma_start(out=rbc, in_=bc_ap)
                    nc.vector.tensor_tensor(
                        out=onT_sb[g][t][pr:pr + 64, nsl],
                        in0=o_un[0:DH, :],
                        in1=rbc,
                        op=mybir.AluOpType.mult,
                    )
                    if filler is not None:
                        filler(h)

            return qk_group, v_phase, attention

        ob_cur = {}

        def proj_group(jp, idx, rep=0):
            nt = jp * 4 + idx // 2
            cc = idx % 2
            if cc == 0:
                ob_cur[jp] = opool.tile([128, C], F16, name=f"ob{rep}_{nt}", tag="ob")
            ob = ob_cur[jp]
            py = ps_mm.tile([128, 512], F32, name=f"y_ps{rep}_{nt}_{cc}", tag="mm")
            for t in range(KT):
                if t < 4:
                    lhsT = onT_sb[0][t][:, nt * 128:(nt + 1) * 128]
                elif t < 8:
                    lhsT = onT_sb[1][t - 4][:, nt * 128:(nt + 1) * 128]
                else:
                    lhsT = ones_sb[:, nt * 128:(nt + 1) * 128]
                nc.tensor.matmul(
                    py,
                    lhsT=lhsT,
                    rhs=wp_sb[t][:, cc * 512:(cc + 1) * 512],
                    start=(t == 0),
                    stop=(t == KT - 1),
                )
            nc.vector.tensor_copy(out=ob[:, cc * 512:(cc + 1) * 512], in_=py)
            if cc == 1:
                nc.sync.dma_start(out=out[nt * 128:(nt + 1) * 128, :], in_=ob)

        # ---- schedule (one body per rep) ----
        for rep in range(reps):
            for k in range(8):
                nc.sync.dma_start(out=xT_sb[k], in_=xT[k * 128:(k + 1) * 128, :])
            nc.sync.dma_start(out=bqk_sb, in_=cbqk[:, :])
            for t in range(KT):
                nc.sync.dma_start(out=wp_sb[t], in_=cwp[t * 128:(t + 1) * 128, :])
            nc.vector.memset(ones_sb, 0.0)
            nc.vector.memset(ones_sb[0:1, :], 1.0)

            qk0, v0, att0 = emit_g(0, rep)
            for j in range(4):
                for ct in range(4, 8):
                    qk0(j, ct)  # K^T g0, all chunks
            v0()
            for ct in range(4):
                qk0(0, ct)

            # g1 closures allocated now so K^T g1 can prefill during att g0
            # (kt pool is double-buffered; weight DMAs defer on WAR)
            qk1, v1, att1 = emit_g(1, rep)
            KQ1 = [(j, ct) for j in range(4) for ct in range(4, 8)]

            def f0(h):
                if h < 4:
                    qk0(1, h, copy_engine="dve")

            def f1(h):
                if h < 4:
                    qk0(2, h, copy_engine="dve")

            def f2(h):
                if h < 4:
                    qk0(3, h, copy_engine="dve")
                else:
                    j, ct = KQ1[h - 4]
                    qk1(j, ct, copy_engine="dve")

            def f3(h):
                lo = 4 + (2 * h if h < 4 else 8 + (h - 4))
                hi = lo + (2 if h < 4 else 1)
                for j, ct in KQ1[lo:hi]:
                    qk1(j, ct, copy_engine="dve")

            att0(0, filler=f0)
            att0(1, filler=f1)
            att0(2, filler=f2)
            att0(3, filler=f3)

            v1()
            for ct in range(4):
                qk1(0, ct)

            def g0(h):
                if h < 4:
                    qk1(1, h, copy_engine="dve")

            def g1f(h):
                if h < 4:
                    qk1(2, h, copy_engine="dve")
                proj_group(0, h, rep)

            def g2f(h):
                if h < 4:
                    qk1(3, h, copy_engine="dve")
                proj_group(1, h, rep)

            att1(0, filler=g0)
            att1(1, filler=g1f)
            att1(2, filler=g2f)
            att1(3, filler=lambda h: proj_group(2, h, rep))
            for idx in range(8):
                proj_group(3, idx, rep)

    _replace_sem_range_clear(nc)
    _split_multi_waits(nc)
    return nc


_NC_CACHE = {}
_NC_KEY = None


def _get_nc(w_qkv, b_qkv, w_proj, b_proj, reps=1):
    global _NC_CACHE, _NC_KEY
    key = hash((
        np.asarray(w_qkv, np.float32).tobytes(),
        np.asarray(b_qkv, np.float32).tobytes(),
        np.asarray(w_proj, np.float32).tobytes(),
        np.asarray(b_proj, np.float32).tobytes(),
    ))
    if _NC_KEY != key:
        _NC_CACHE = {}
        _NC_KEY = key
    if reps not in _NC_CACHE:
        consts = _make_consts(w_qkv, b_qkv, w_proj, b_proj)
        _NC_CACHE[reps] = build_bass(consts, reps=reps)
    return _NC_CACHE[reps]


def make_in_maps(x):
    x = np.asarray(x, np.float32)
    return [
        {"xT": np.ascontiguousarray(x[b].T).astype(NPDT)} for b in range(NCORES)
    ]


def assemble_output(results):
    return np.stack(
        [np.asarray(r["out"], np.float32) for r in results]
    )


_RUNNER_CACHE = {}


def _get_runner(nc):
    """Build (once) a jitted shard_map executor for this nc. Repeat kernel()
    calls reuse the compiled executable; only x is re-uploaded per call."""
    key = id(nc)
    if key in _RUNNER_CACHE:
        return _RUNNER_CACHE[key]

    import jax
    from jax.sharding import Mesh, PartitionSpec
    from jax.experimental.shard_map import shard_map
    from concourse import bass2jax as b2j

    b2j.install_neuronx_cc_hook()
    partition_name = nc.partition_id_tensor.name if nc.partition_id_tensor else None
    in_names, out_names, out_avals, zero_outs = [], [], [], []
    for alloc in nc.m.functions[0].allocations:
        if not isinstance(alloc, mybir.MemoryLocationSet):
            continue
        name = alloc.memorylocations[0].name
        if alloc.kind == "ExternalInput":
            if name != partition_name:
                in_names.append(name)
        elif alloc.kind == "ExternalOutput":
            out_avals.append(
                jax.core.ShapedArray(
                    tuple(alloc.tensor_shape), mybir.dt.np(alloc.dtype)
                )
            )
            zero_outs.append(np.zeros(alloc.tensor_shape, mybir.dt.np(alloc.dtype)))
            out_names.append(name)
    n_params = len(in_names)
    n_outs = len(out_names)
    all_in_names = list(in_names) + list(out_names)
    if partition_name is not None:
        all_in_names.append(partition_name)

    def _body(*args):
        operands = list(args)
        if partition_name is not None:
            operands.append(b2j.partition_id_tensor())
        outs = b2j._bass_exec_p.bind(
            *operands,
            out_avals=tuple(out_avals),
            in_names=tuple(all_in_names),
            out_names=tuple(out_names),
            lowering_input_output_aliases=(),
            sim_require_finite=True,
            sim_require_nnan=True,
            nc=nc,
        )
        return tuple(outs)

    devices = jax.devices()[:NCORES]
    mesh = Mesh(np.asarray(devices), ("core",))
    sharded = jax.jit(
        shard_map(
            _body,
            mesh=mesh,
            in_specs=(PartitionSpec("core"),) * (n_params + n_outs),
            out_specs=(PartitionSpec("core"),) * n_outs,
            check_rep=False,
        ),
        keep_unused=True,
    )
    concat_zeros = [
        np.zeros((NCORES * z.shape[0], *z.shape[1:]), z.dtype) for z in zero_outs
    ]

    def runner(in_maps):
        per_core = [[np.asarray(m[n]) for n in in_names] for m in in_maps]
        concat_in = [
            np.concatenate([per_core[c][i] for c in range(NCORES)], axis=0)
            for i in range(n_params)
        ]
        out = sharded(*concat_in, *concat_zeros)
        outs_np = np.asarray(out[0]).reshape(NCORES, *out_avals[0].shape)
        return outs_np

    _RUNNER_CACHE[key] = runner
    return runner


def run(x, w_qkv, b_qkv, w_proj, b_proj, **_ignored):
    nc = _get_nc(w_qkv, b_qkv, w_proj, b_proj)
    runner = _get_runner(nc)
    outs = runner(make_in_maps(x))
    return outs.astype(np.float32), None


def kernel(x, w_qkv, b_qkv, w_proj, b_proj):
    out, _ = run(x, w_qkv, b_qkv, w_proj, b_proj)
    return out
